# revision 29
# baseline (speedup 1.0000x reference)
"""Two-layer GAT on 8 Trainium2 NeuronCores (Bass/Tile SPMD kernel).

Sharding: nodes are range-partitioned across the 8 cores; each core owns the
edges whose *destination* falls in its partition (segment softmax is per-dst,
so the softmax/aggregation is fully core-local). Each layer's node feature
table ([h | a_src], bf16) is computed sharded (each core does its own nodes)
and AllGathered in two chunks (so the first chunk's transfer overlaps the
producer's tail); the a_dst table stays core-local since every edge's dst is
owned by the core that processes it.

Per-edge math uses the shift-invariance of softmax:
    out_d = sum_e exp(alpha_e) * h[src_e] / sum_e exp(alpha_e)
so a single pass over the edges suffices (no segment max needed; alpha is
O(1) here so exp cannot overflow).
"""

import sys

sys.path.insert(0, "/opt/trn_rl_repo")

import numpy as np

try:
    from ml_dtypes import bfloat16 as np_bf16
    from ml_dtypes import float8_e4m3 as np_fp8
except ImportError:
    import jax.numpy as _jnp
    np_bf16 = _jnp.bfloat16
    np_fp8 = _jnp.float8_e4m3

# ---------------------------------------------------------------------------
# configuration
# ---------------------------------------------------------------------------

FULL_CFG = dict(
    N=100000,      # real nodes
    IN_CH=512,     # input features
    HEADS=8,
    C=16,          # out channels per head
    NC=8,          # cores
)

NEG_SLOPE = 0.2
EPS = 1e-16


def _derive(cfg):
    d = dict(cfg)
    d["HC"] = d["HEADS"] * d["C"]                 # 128
    assert d["HC"] == 128
    assert d["IN_CH"] % 128 == 0
    d["KC"] = d["IN_CH"] // 128                   # k-chunks for x@W1
    assert d["N"] % d["NC"] == 0
    d["OWN"] = d["N"] // d["NC"]                  # real nodes per core
    d["BLK"] = (d["OWN"] + 127) // 128            # dst blocks per core
    assert d["BLK"] % 2 == 0
    d["OWN_PAD"] = d["BLK"] * 128
    d["NP"] = d["NC"] * d["OWN_PAD"]              # padded global nodes
    # AllGather chunking: chunk 0 = first CB0 blocks, chunk 1 = the rest
    d["CB0"] = 48                                 # 4-group & pair aligned
    d["CH0"] = d["CB0"] * 128                     # rows in chunk 0
    d["CH1"] = d["OWN_PAD"] - d["CH0"]
    return d


# ---------------------------------------------------------------------------
# host-side prep: edge partitioning / padding / layouts
# ---------------------------------------------------------------------------

def _host_prep(cfg, x, edge_index, W1, att_src1, att_dst1, bias1, W2,
               att_src2, att_dst2, bias2):
    N, NC, OWN, BLK, OWN_PAD, NP, KC, H, C = (
        cfg["N"], cfg["NC"], cfg["OWN"], cfg["BLK"], cfg["OWN_PAD"],
        cfg["NP"], cfg["KC"], cfg["HEADS"], cfg["C"])
    HC = H * C
    CH0, CH1 = cfg["CH0"], cfg["CH1"]

    src = np.asarray(edge_index[0], dtype=np.int64)
    dst = np.asarray(edge_index[1], dtype=np.int64)

    core = dst // OWN                       # owning core of each edge
    ldst = (dst - core * OWN).astype(np.int64)    # local dst id [0, OWN)
    # table row id under the 2-chunk AllGather layout:
    #   chunk 0: rows [0, NC*CH0) = concat over cores of local [0, CH0)
    #   chunk 1: rows [NC*CH0, NP) = concat over cores of local [CH0, OWN_PAD)
    s_core_id = src // OWN
    s_local = src % OWN
    srcp = np.where(
        s_local < CH0,
        s_core_id * CH0 + s_local,
        NC * CH0 + s_core_id * CH1 + (s_local - CH0)).astype(np.int32)
    blk = ldst // 128
    slot = ldst % 128

    # per-(core, block) edge counts -> shared tile counts T_b (SPMD uniform)
    counts = np.zeros((NC, BLK), dtype=np.int64)
    np.add.at(counts, (core, blk), 1)
    Tb = np.maximum(1, (counts.max(axis=0) + 127) // 128).astype(np.int64)
    off = np.zeros(BLK, dtype=np.int64)
    off[1:] = np.cumsum(Tb)[:-1]
    totT = int(Tb.sum())

    # per-core edge arrays, laid out [128, totT]:
    #   edge e of block b  ->  (partition p = e%128, column off[b] + e//128)
    srcp_a = np.zeros((NC, 128, totT), dtype=np.int32)
    dstl_a = np.zeros((NC, 128, totT), dtype=np.int32)
    # static one-hot scatter matrices Pm[p, col, d] = (slot == d), bf16
    Pm_a = np.zeros((NC, 128, totT, 128), dtype=np_fp8)

    order = np.lexsort((blk, core))
    s_core, s_blk = core[order], blk[order]
    s_srcp, s_ldst, s_slot = srcp[order], ldst[order], slot[order]
    grp = s_core * BLK + s_blk
    first = np.ones(len(grp), dtype=bool)
    first[1:] = grp[1:] != grp[:-1]
    starts = np.flatnonzero(first)
    group_start = np.repeat(starts, np.diff(np.append(starts, len(grp))))
    pos = np.arange(len(grp)) - group_start
    pp = pos % 128
    col = off[s_blk] + pos // 128
    srcp_a[s_core, pp, col] = s_srcp
    dstl_a[s_core, pp, col] = s_ldst
    Pm_a[s_core, pp, col, s_slot] = np_fp8(1.0)

    xf = np.asarray(x, np.float32)
    attS1 = np.broadcast_to(
        np.asarray(att_src1, np.float32).reshape(1, HC).astype(np_bf16),
        (128, HC)).copy()
    attD1 = np.broadcast_to(
        np.asarray(att_dst1, np.float32).reshape(1, HC).astype(np_bf16),
        (128, HC)).copy()
    b1b = np.broadcast_to(
        np.asarray(bias1, np.float32).reshape(1, HC), (128, HC)).copy()
    W1b = np.ascontiguousarray(
        np.asarray(W1, np.float32).reshape(KC, 128, HC).transpose(1, 0, 2)
    ).astype(np_fp8)
    W2f = np.asarray(W2, np.float32)
    va = np.einsum("khc,hc->kh", W2f.reshape(HC, H, C),
                   np.asarray(att_src2, np.float32))
    vd = np.einsum("khc,hc->kh", W2f.reshape(HC, H, C),
                   np.asarray(att_dst2, np.float32))
    vavd = np.ascontiguousarray(
        np.concatenate([va, vd], axis=1)).astype(np_bf16)  # [128, 2H]
    b2b = np.broadcast_to(
        np.asarray(bias2, np.float32).reshape(1, C), (128, C)).copy()

    W2s = W2f.reshape(HC, H, C).mean(axis=1)          # head-mean folded in
    shared = dict(W1b=W1b, attS1=attS1, attD1=attD1,
                  b1b=b1b, W2=np.ascontiguousarray(W2s).astype(np_bf16),
                  vavd=vavd, b2b=b2b)
    in_maps = []
    for m in range(NC):
        im = dict(shared)
        xp = np.zeros((OWN_PAD, cfg["IN_CH"]), dtype=np.float32)
        xp[:OWN] = xf[m * OWN:(m + 1) * OWN]
        im["xTb"] = np.ascontiguousarray(
            xp.reshape(OWN_PAD, KC, 128).transpose(2, 1, 0)).astype(np_fp8)
        im["sd_a"] = np.ascontiguousarray(
            np.concatenate([srcp_a[m], dstl_a[m]], axis=1))
        im["Pm_a"] = Pm_a[m].reshape(128, totT * 128)
        in_maps.append(im)

    return in_maps, Tb.tolist(), off.tolist(), totT


# ---------------------------------------------------------------------------
# device program
# ---------------------------------------------------------------------------

def build_program(cfg, Tb, off, totT):
    from concourse import bacc, bass, mybir, tile
    from concourse.masks import make_identity

    f32 = mybir.dt.float32
    bf16 = mybir.dt.bfloat16
    fp8 = mybir.dt.float8e4
    i32 = mybir.dt.int32
    X = mybir.AxisListType.X
    AF = mybir.ActivationFunctionType
    NC, NP, OWN_PAD, BLK, KC, H, C = (
        cfg["NC"], cfg["NP"], cfg["OWN_PAD"], cfg["BLK"], cfg["KC"],
        cfg["HEADS"], cfg["C"])
    HC = H * C
    W = 8 + HC               # table row width: [h (128) | a_src (8)]
    CB0, CH0, CH1 = cfg["CB0"], cfg["CH0"], cfg["CH1"]

    nc = bacc.Bacc("TRN2", target_bir_lowering=False, debug=False,
                   num_devices=NC)

    # inputs
    t_xTb = nc.dram_tensor("xTb", [128, KC, OWN_PAD], fp8,
                           kind="ExternalInput")
    t_W1b = nc.dram_tensor("W1b", [128, KC, HC], fp8, kind="ExternalInput")
    t_attS1 = nc.dram_tensor("attS1", [128, HC], bf16, kind="ExternalInput")
    t_attD1 = nc.dram_tensor("attD1", [128, HC], bf16, kind="ExternalInput")
    t_b1b = nc.dram_tensor("b1b", [128, HC], f32, kind="ExternalInput")
    t_W2 = nc.dram_tensor("W2", [HC, C], bf16, kind="ExternalInput")
    t_vavd = nc.dram_tensor("vavd", [HC, 2 * H], bf16, kind="ExternalInput")
    t_b2b = nc.dram_tensor("b2b", [128, C], f32, kind="ExternalInput")
    t_sd = nc.dram_tensor("sd_a", [128, 2 * totT], i32, kind="ExternalInput")
    t_Pm = nc.dram_tensor("Pm_a", [128, totT * 128], fp8,
                          kind="ExternalInput")
    t_out = nc.dram_tensor("out", [OWN_PAD, C], f32, kind="ExternalOutput")

    def fv(ap, dims, extra_offset=0):
        """View `ap` with custom free-dim [step, count] pairs."""
        return bass.AP(ap.tensor, ap.offset + extra_offset, [ap.ap[0]] + dims)

    with tile.TileContext(nc) as tc:
        with (
            tc.tile_pool(name="const", bufs=1) as cpool,
            tc.tile_pool(name="dram", bufs=1, space="DRAM") as dram,
        ):
            # ---------------- constants ----------------
            W1_sb = cpool.tile([128, KC * HC], fp8, tag="w1")
            nc.sync.dma_start(
                out=fv(W1_sb[:], [[HC, KC], [1, HC]]), in_=t_W1b[:, :, :])
            attS1_sb = cpool.tile([128, HC], bf16, tag="attS")
            nc.sync.dma_start(out=attS1_sb[:], in_=t_attS1[:, :])
            attD1_sb = cpool.tile([128, HC], bf16, tag="attD")
            nc.sync.dma_start(out=attD1_sb[:], in_=t_attD1[:, :])
            b1_sb = cpool.tile([128, HC], f32, tag="b1")
            nc.sync.dma_start(out=b1_sb[:], in_=t_b1b[:, :])
            W2_sb = cpool.tile([HC, C], bf16, tag="w2")
            nc.sync.dma_start(out=W2_sb[:], in_=t_W2[:, :])
            vavd_sb = cpool.tile([HC, 2 * H], bf16, tag="vavd")
            nc.sync.dma_start(out=vavd_sb[:], in_=t_vavd[:, :])
            b2_sb = cpool.tile([128, C], f32, tag="b2")
            nc.sync.dma_start(out=b2_sb[:], in_=t_b2b[:, :])
            identb = cpool.tile([128, 128], bf16, tag="identb")
            make_identity(nc, identb[:])
            # final logits staging for the batched log_softmax pass
            redall = cpool.tile([128, BLK * C], f32, tag="redall")

            # internal DRAM
            h1own = dram.tile([OWN_PAD, W], bf16, tag="h1own")
            adst1 = dram.tile([OWN_PAD, H], f32, tag="adst1")
            table1 = dram.tile([NP, W], bf16, tag="table1")
            h2own = dram.tile([OWN_PAD, W], bf16, tag="h2own")
            adst2 = dram.tile([OWN_PAD, H], f32, tag="adst2")
            table2 = dram.tile([NP, W], bf16, tag="table2")

            def allgather_chunk(src_t, dst_t, c):
                # chunk 0 is issued mid-producer so its transfer overlaps the
                # producer's tail; chunk 1 goes right after the last block
                lo, hi = (0, CH0) if c == 0 else (CH0, OWN_PAD)
                glo, ghi = (0, NC * CH0) if c == 0 else (NC * CH0, NP)
                nc.gpsimd.collective_compute(
                    "AllGather", mybir.AluOpType.bypass,
                    replica_groups=[list(range(NC))],
                    ins=[src_t[lo:hi, :].opt()],
                    outs=[dst_t[glo:ghi, :].opt()])

            # ------- phase A: own-shard h1 table (h = x@W1, attn logits) ---
            # processed in groups of 4 128-row blocks to cut issue overhead
            with (
                tc.tile_pool(name="pa", bufs=4) as pa,
                tc.tile_pool(name="pa_ps", bufs=2, space="PSUM") as pa_ps,
            ):
                i0 = 0
                while i0 < BLK:
                    nb = min(4, BLK - i0)
                    r0 = i0 * 128
                    R = nb * 128
                    xt = pa.tile([128, KC * R], fp8, tag=f"xt{nb}")
                    nc.sync.dma_start(
                        out=fv(xt[:], [[R, KC], [1, R]]),
                        in_=t_xTb[:, :, r0:r0 + R])
                    hbp = pa.tile([128, nb * W], bf16, tag=f"hbp{nb}")
                    for h in range(nb):
                        ph = pa_ps.tile([128, HC], f32, tag=f"ph{h}")
                        for k in range(KC):
                            nc.tensor.matmul(
                                out=ph[:],
                                lhsT=xt[:, k * R + h * 128:
                                        k * R + (h + 1) * 128],
                                rhs=W1_sb[:, k * HC:(k + 1) * HC],
                                start=(k == 0), stop=(k == KC - 1))
                        nc.scalar.copy(out=hbp[:, h * W:h * W + HC], in_=ph[:])
                    # attention logits from the bf16 h (same as gathers see)
                    tmpS = pa.tile([128, nb * HC], bf16, tag=f"tmpS{nb}")
                    nc.vector.tensor_tensor(
                        out=fv(tmpS[:], [[HC, nb], [1, HC]]),
                        in0=fv(hbp[:], [[W, nb], [1, HC]]),
                        in1=fv(attS1_sb[:], [[0, nb], [1, HC]]),
                        op=mybir.AluOpType.mult)
                    asr = pa.tile([128, nb * H], f32, tag=f"asr{nb}")
                    nc.vector.reduce_sum(
                        out=asr[:], in_=fv(tmpS[:], [[C, nb * H], [1, C]]),
                        axis=X)
                    nc.scalar.copy(
                        out=fv(hbp[:], [[W, nb], [1, H]], extra_offset=HC),
                        in_=asr[:])
                    tmpD = pa.tile([128, nb * HC], bf16, tag=f"tmpD{nb}")
                    nc.vector.tensor_tensor(
                        out=fv(tmpD[:], [[HC, nb], [1, HC]]),
                        in0=fv(hbp[:], [[W, nb], [1, HC]]),
                        in1=fv(attD1_sb[:], [[0, nb], [1, HC]]),
                        op=mybir.AluOpType.mult)
                    adt = pa.tile([128, nb * H], f32, tag=f"adt{nb}")
                    nc.vector.reduce_sum(
                        out=adt[:], in_=fv(tmpD[:], [[C, nb * H], [1, C]]),
                        axis=X)
                    h1ap = h1own[:]
                    nc.sync.dma_start(
                        out=bass.AP(h1ap.tensor, h1ap.offset + r0 * W,
                                    [[W, 128], [128 * W, nb], [1, W]]),
                        in_=fv(hbp[:], [[W, nb], [1, W]]))
                    a1ap = adst1[:]
                    nc.sync.dma_start(
                        out=bass.AP(a1ap.tensor, a1ap.offset + r0 * H,
                                    [[H, 128], [128 * H, nb], [1, H]]),
                        in_=fv(adt[:], [[H, nb], [1, H]]))
                    i0 += nb
                    if i0 == CB0:
                        allgather_chunk(h1own, table1, 0)

            allgather_chunk(h1own, table1, 1)

            # ---------------- edge phase (shared between layers) ----------
            # pairs of dst blocks per iteration: one gather / one one-hot /
            # one weighting op per pair, two PSUM scatter groups
            def edge_phase(tag, tab, adst_tab, finish, mid_hook=None):
                with (
                    tc.tile_pool(name=f"eg{tag}", bufs=4) as eg,
                    tc.tile_pool(name=f"epm{tag}", bufs=6) as epm,
                    tc.tile_pool(name=f"ew{tag}", bufs=3) as ew,
                    tc.tile_pool(name=f"ef{tag}", bufs=3) as ef,
                    tc.tile_pool(name=f"eps{tag}", bufs=2, space="PSUM") as eps,
                    tc.tile_pool(name=f"fps{tag}", bufs=2, space="PSUM") as fps,
                ):
                    for ip in range(BLK // 2):
                        b0 = 2 * ip
                        T0, T1 = Tb[b0], Tb[b0 + 1]
                        TP = T0 + T1
                        o = off[b0]
                        idxd = eg.tile([128, 2 * TP], i32, tag="idxd")
                        nc.sync.dma_start(
                            out=idxd[:],
                            in_=fv(t_sd[:, 0:1], [[totT, 2], [1, TP]],
                                   extra_offset=o))

                        gath = eg.tile([128, TP * W], bf16, tag="gath")
                        nc.gpsimd.indirect_dma_start(
                            out=gath[:], out_offset=None,
                            in_=tab[:, :],
                            in_offset=bass.IndirectOffsetOnAxis(
                                ap=idxd[:, 0:TP], axis=0))
                        gd = eg.tile([128, TP * H], f32, tag="gd")
                        nc.gpsimd.indirect_dma_start(
                            out=gd[:], out_offset=None,
                            in_=adst_tab[:, :],
                            in_offset=bass.IndirectOffsetOnAxis(
                                ap=idxd[:, TP:2 * TP], axis=0))

                        # static one-hot Pm[e, (j, d)] streamed from DRAM
                        Pm = epm.tile([128, TP * 128], fp8, tag="Pm")
                        nc.sync.dma_start(
                            out=Pm[:], in_=t_Pm[:, o * 128:(o + TP) * 128])

                        # alpha = a_src[src] + a_dst[dst]; ex = exp(lrelu(.))
                        ax = ef.tile([128, TP * H], f32, tag="ax")
                        nc.vector.tensor_add(
                            out=ax[:],
                            in0=fv(gath[:], [[W, TP], [1, H]],
                                   extra_offset=HC),
                            in1=gd[:])
                        nc.vector.scalar_tensor_tensor(
                            out=ax[:], in0=ax[:], scalar=NEG_SLOPE,
                            in1=ax[:], op0=mybir.AluOpType.mult,
                            op1=mybir.AluOpType.max)
                        ex = ef.tile([128, TP * H], f32, tag="ex")
                        nc.scalar.activation(out=ex[:], in_=ax[:], func=AF.Exp)

                        # weighted features (+ ex appended per tile): [T, 136]
                        wt = ew.tile([128, TP * W], bf16, tag="wt")
                        nc.vector.tensor_tensor(
                            out=fv(wt[:], [[W, TP], [C, H], [1, C]]),
                            in0=fv(gath[:], [[W, TP], [C, H], [1, C]]),
                            in1=fv(ex[:], [[H, TP], [1, H], [0, C]]),
                            op=mybir.AluOpType.mult)
                        nc.scalar.copy(
                            out=fv(wt[:], [[W, TP], [1, H]], extra_offset=HC),
                            in_=ex[:])

                        for h, (lo, hi) in enumerate(((0, T0), (T0, TP))):
                            nd = eps.tile([128, W], f32, tag=f"nd{h}")
                            for j in range(lo, hi):
                                nc.tensor.matmul(
                                    out=nd[:],
                                    lhsT=Pm[:, j * 128:(j + 1) * 128],
                                    rhs=wt[:, j * W:(j + 1) * W],
                                    start=(j == lo), stop=(j == hi - 1))
                            finish(b0 + h, nd, ef, fps)
                        if mid_hook is not None and ip == CB0 // 2 - 1:
                            mid_hook()

            # ---------------- layer-1 block finisher ----------------------
            def finish1(b, nd, ef, fps):
                dr = ef.tile([128, H], f32, tag="dr")
                nc.vector.tensor_scalar_add(dr[:], nd[:, HC:W], EPS)
                nc.vector.reciprocal(out=dr[:], in_=dr[:])
                g = ef.tile([128, HC], f32, tag="g")
                nc.vector.tensor_tensor(
                    out=fv(g[:], [[C, H], [1, C]]),
                    in0=fv(nd[:], [[C, H], [1, C]]),
                    in1=fv(dr[:], [[1, H], [0, C]]),
                    op=mybir.AluOpType.mult)
                nc.vector.tensor_add(out=g[:], in0=g[:], in1=b1_sb[:])
                # ELU
                tn = ef.tile([128, HC], f32, tag="tn")
                nc.vector.tensor_scalar_min(tn[:], g[:], 0.0)
                te = ef.tile([128, HC], f32, tag="te")
                nc.scalar.activation(out=te[:], in_=tn[:], func=AF.Exp)
                nc.vector.scalar_tensor_tensor(
                    out=g[:], in0=g[:], scalar=0.0, in1=te[:],
                    op0=mybir.AluOpType.max, op1=mybir.AluOpType.add)
                # pack h2 row [ELU | a_src2] (bf16); the ELU "-1" rides the
                # cast's bias so it costs no extra op
                hb2 = ef.tile([128, W], bf16, tag="hb2")
                nc.scalar.activation(out=hb2[:, :HC], in_=g[:], func=AF.Copy,
                                     bias=-1.0)
                # a_src2 / a_dst2 via g @ (W2 @ att2): needs gT as lhsT
                gtp = fps.tile([128, 128], bf16, tag="gtp")
                nc.tensor.transpose(out=gtp[:], in_=hb2[:, :HC],
                                    identity=identb[:])
                gts = ef.tile([128, 128], bf16, tag="gts")
                nc.scalar.copy(out=gts[:], in_=gtp[:])
                a2p = fps.tile([128, 2 * H], f32, tag="a2p")
                nc.tensor.matmul(out=a2p[:], lhsT=gts[:], rhs=vavd_sb[:],
                                 start=True, stop=True)
                a2s = ef.tile([128, 2 * H], f32, tag="a2s")
                nc.scalar.copy(out=a2s[:], in_=a2p[:])
                nc.scalar.copy(out=hb2[:, HC:W], in_=a2s[:, :H])
                nc.sync.dma_start(
                    out=h2own[b * 128:(b + 1) * 128, :], in_=hb2[:])
                nc.sync.dma_start(
                    out=adst2[b * 128:(b + 1) * 128, :], in_=a2s[:, H:])

            # ---------------- layer-2 block finisher ----------------------
            # (log_softmax deferred to one batched pass -> no per-block
            #  activation-table thrash between EXP and LN)
            def finish2(b, nd, ef, fps):
                dr = ef.tile([128, H], f32, tag="dr")
                nc.vector.tensor_scalar_add(dr[:], nd[:, HC:W], EPS)
                nc.vector.reciprocal(out=dr[:], in_=dr[:])
                g = ef.tile([128, HC], bf16, tag="g")
                nc.vector.tensor_tensor(
                    out=fv(g[:], [[C, H], [1, C]]),
                    in0=fv(nd[:], [[C, H], [1, C]]),
                    in1=fv(dr[:], [[1, H], [0, C]]),
                    op=mybir.AluOpType.mult)
                atp = fps.tile([128, 128], bf16, tag="gtp")
                nc.tensor.transpose(out=atp[:], in_=g[:], identity=identb[:])
                ats = ef.tile([128, 128], bf16, tag="gts")
                nc.scalar.copy(out=ats[:], in_=atp[:])
                o2 = fps.tile([128, C], f32, tag="o2")
                nc.tensor.matmul(out=o2[:], lhsT=ats[:], rhs=W2_sb[:],
                                 start=True, stop=True)
                nc.scalar.copy(out=redall[:, b * C:(b + 1) * C], in_=o2[:])

            # ---------------- run both layers ------------------------------
            edge_phase("1", table1, adst1, finish1,
                       mid_hook=lambda: allgather_chunk(h2own, table2, 0))

            allgather_chunk(h2own, table2, 1)

            edge_phase("2", table2, adst2, finish2)

            # ---------------- batched bias + log_softmax ------------------
            with tc.tile_pool(name="fin", bufs=1) as fin:
                nc.vector.tensor_add(
                    out=fv(redall[:], [[C, BLK], [1, C]]),
                    in0=fv(redall[:], [[C, BLK], [1, C]]),
                    in1=fv(b2_sb[:], [[0, BLK], [1, C]]))
                mx = fin.tile([128, BLK], f32, tag="mx")
                nc.vector.reduce_max(
                    out=mx[:], in_=fv(redall[:], [[C, BLK], [1, C]]), axis=X)
                nc.vector.tensor_tensor(
                    out=fv(redall[:], [[C, BLK], [1, C]]),
                    in0=fv(redall[:], [[C, BLK], [1, C]]),
                    in1=fv(mx[:], [[1, BLK], [0, C]]),
                    op=mybir.AluOpType.subtract)
                es = fin.tile([128, BLK * C], f32, tag="es")
                nc.scalar.activation(out=es[:], in_=redall[:], func=AF.Exp)
                sm = fin.tile([128, BLK], f32, tag="sm")
                nc.vector.reduce_sum(
                    out=sm[:], in_=fv(es[:], [[C, BLK], [1, C]]), axis=X)
                ls = fin.tile([128, BLK], f32, tag="ls")
                nc.scalar.activation(out=ls[:], in_=sm[:], func=AF.Ln)
                nc.vector.tensor_tensor(
                    out=fv(redall[:], [[C, BLK], [1, C]]),
                    in0=fv(redall[:], [[C, BLK], [1, C]]),
                    in1=fv(ls[:], [[1, BLK], [0, C]]),
                    op=mybir.AluOpType.subtract)
                oap = t_out[:, :]
                nc.sync.dma_start(
                    out=bass.AP(oap.tensor, oap.offset,
                                [[C, 128], [128 * C, BLK], [1, C]]),
                    in_=redall[:])

    nc.compile()
    return nc


# ---------------------------------------------------------------------------
# entry point
# ---------------------------------------------------------------------------

def _run(cfg, inputs, trace=False):
    from concourse.bass_utils import run_bass_kernel_spmd

    cfg = _derive(cfg)
    in_maps, Tb, off, totT = _host_prep(cfg, **inputs)
    nc = build_program(cfg, Tb, off, totT)
    res = run_bass_kernel_spmd(
        nc, in_maps, core_ids=list(range(cfg["NC"])), trace=trace)
    outs = []
    for m in range(cfg["NC"]):
        outs.append(res.results[m]["out"][:cfg["OWN"]])
    full = np.concatenate(outs, axis=0)
    return full, res


def kernel(x, edge_index, W1, att_src1, att_dst1, bias1, W2, att_src2,
           att_dst2, bias2):
    inputs = dict(x=np.asarray(x, np.float32),
                  edge_index=np.asarray(edge_index),
                  W1=W1, att_src1=att_src1, att_dst1=att_dst1, bias1=bias1,
                  W2=W2, att_src2=att_src2, att_dst2=att_dst2, bias2=bias2)
    out, _ = _run(FULL_CFG, inputs, trace=False)
    return out


# revision 30
# speedup vs baseline: 1.2327x; 1.2327x over previous
"""Two-layer GAT on 8 Trainium2 NeuronCores (Bass/Tile SPMD kernel).

Sharding: nodes are range-partitioned across the 8 cores; each core owns the
edges whose *destination* falls in its partition (segment softmax is per-dst,
so the softmax/aggregation is fully core-local). Each layer's node feature
table ([h | a_src], bf16) is computed sharded (each core does its own nodes)
and AllGathered in two chunks (so the first chunk's transfer overlaps the
producer's tail); the a_dst table stays core-local since every edge's dst is
owned by the core that processes it.

Per-edge math uses the shift-invariance of softmax:
    out_d = sum_e exp(alpha_e) * h[src_e] / sum_e exp(alpha_e)
so a single pass over the edges suffices (no segment max needed; alpha is
O(1) here so exp cannot overflow).
"""

import sys

sys.path.insert(0, "/opt/trn_rl_repo")

import numpy as np

try:
    from ml_dtypes import bfloat16 as np_bf16
    from ml_dtypes import float8_e4m3 as np_fp8
except ImportError:
    import jax.numpy as _jnp
    np_bf16 = _jnp.bfloat16
    np_fp8 = _jnp.float8_e4m3

# ---------------------------------------------------------------------------
# configuration
# ---------------------------------------------------------------------------

FULL_CFG = dict(
    N=100000,      # real nodes
    IN_CH=512,     # input features
    HEADS=8,
    C=16,          # out channels per head
    NC=8,          # cores
)

NEG_SLOPE = 0.2
EPS = 1e-16


def _derive(cfg):
    d = dict(cfg)
    d["HC"] = d["HEADS"] * d["C"]                 # 128
    assert d["HC"] == 128
    assert d["IN_CH"] % 128 == 0
    d["KC"] = d["IN_CH"] // 128                   # k-chunks for x@W1
    assert d["N"] % d["NC"] == 0
    d["OWN"] = d["N"] // d["NC"]                  # real nodes per core
    d["BLK"] = (d["OWN"] + 127) // 128            # dst blocks per core
    assert d["BLK"] % 2 == 0
    d["OWN_PAD"] = d["BLK"] * 128
    d["NP"] = d["NC"] * d["OWN_PAD"]              # padded global nodes
    # AllGather chunking: chunk 0 = first CB0 blocks, chunk 1 = the rest
    d["CB0"] = (d["BLK"] // 2 + 1) // 2 * 2       # 50 blocks (pair-aligned)
    d["CH0"] = d["CB0"] * 128                     # rows in chunk 0
    d["CH1"] = d["OWN_PAD"] - d["CH0"]
    return d


# ---------------------------------------------------------------------------
# host-side prep: edge partitioning / padding / layouts
# ---------------------------------------------------------------------------

def _host_prep(cfg, x, edge_index, W1, att_src1, att_dst1, bias1, W2,
               att_src2, att_dst2, bias2):
    N, NC, OWN, BLK, OWN_PAD, NP, KC, H, C = (
        cfg["N"], cfg["NC"], cfg["OWN"], cfg["BLK"], cfg["OWN_PAD"],
        cfg["NP"], cfg["KC"], cfg["HEADS"], cfg["C"])
    HC = H * C
    CH0, CH1 = cfg["CH0"], cfg["CH1"]

    src = np.asarray(edge_index[0], dtype=np.int64)
    dst = np.asarray(edge_index[1], dtype=np.int64)

    core = dst // OWN                       # owning core of each edge
    ldst = (dst - core * OWN).astype(np.int64)    # local dst id [0, OWN)
    srcp = ((src // OWN) * OWN_PAD + (src % OWN)).astype(np.int32)  # padded gid
    blk = ldst // 128
    slot = ldst % 128

    # per-(core, block) edge counts -> shared tile counts T_b (SPMD uniform)
    counts = np.zeros((NC, BLK), dtype=np.int64)
    np.add.at(counts, (core, blk), 1)
    Tb = np.maximum(1, (counts.max(axis=0) + 127) // 128).astype(np.int64)
    off = np.zeros(BLK, dtype=np.int64)
    off[1:] = np.cumsum(Tb)[:-1]
    totT = int(Tb.sum())

    # per-core edge arrays, laid out [128, totT]:
    #   edge e of block b  ->  (partition p = e%128, column off[b] + e//128)
    srcp_a = np.zeros((NC, 128, totT), dtype=np.int32)
    dstl_a = np.zeros((NC, 128, totT), dtype=np.int32)
    # static one-hot scatter matrices Pm[p, col, d] = (slot == d), bf16
    Pm_a = np.zeros((NC, 128, totT, 128), dtype=np_fp8)

    order = np.lexsort((blk, core))
    s_core, s_blk = core[order], blk[order]
    s_srcp, s_ldst, s_slot = srcp[order], ldst[order], slot[order]
    grp = s_core * BLK + s_blk
    first = np.ones(len(grp), dtype=bool)
    first[1:] = grp[1:] != grp[:-1]
    starts = np.flatnonzero(first)
    group_start = np.repeat(starts, np.diff(np.append(starts, len(grp))))
    pos = np.arange(len(grp)) - group_start
    pp = pos % 128
    col = off[s_blk] + pos // 128
    srcp_a[s_core, pp, col] = s_srcp
    dstl_a[s_core, pp, col] = s_ldst
    Pm_a[s_core, pp, col, s_slot] = np_fp8(1.0)

    xf = np.asarray(x, np.float32)
    attS1 = np.broadcast_to(
        np.asarray(att_src1, np.float32).reshape(1, HC).astype(np_bf16),
        (128, HC)).copy()
    attD1 = np.broadcast_to(
        np.asarray(att_dst1, np.float32).reshape(1, HC).astype(np_bf16),
        (128, HC)).copy()
    b1b = np.broadcast_to(
        np.asarray(bias1, np.float32).reshape(1, HC), (128, HC)).copy()
    W1b = np.ascontiguousarray(
        np.asarray(W1, np.float32).reshape(KC, 128, HC).transpose(1, 0, 2)
    ).astype(np_fp8)
    W2f = np.asarray(W2, np.float32)
    va = np.einsum("khc,hc->kh", W2f.reshape(HC, H, C),
                   np.asarray(att_src2, np.float32))
    vd = np.einsum("khc,hc->kh", W2f.reshape(HC, H, C),
                   np.asarray(att_dst2, np.float32))
    vavd = np.ascontiguousarray(
        np.concatenate([va, vd], axis=1)).astype(np_bf16)  # [128, 2H]
    b2b = np.broadcast_to(
        np.asarray(bias2, np.float32).reshape(1, C), (128, C)).copy()

    W2s = W2f.reshape(HC, H, C).mean(axis=1)          # head-mean folded in
    shared = dict(W1b=W1b, attS1=attS1, attD1=attD1,
                  b1b=b1b, W2=np.ascontiguousarray(W2s).astype(np_bf16),
                  vavd=vavd, b2b=b2b)
    in_maps = []
    for m in range(NC):
        im = dict(shared)
        xp = np.zeros((OWN_PAD, cfg["IN_CH"]), dtype=np.float32)
        xp[:OWN] = xf[m * OWN:(m + 1) * OWN]
        im["xTb"] = np.ascontiguousarray(
            xp.reshape(OWN_PAD, KC, 128).transpose(2, 1, 0)).astype(np_fp8)
        im["sd_a"] = np.ascontiguousarray(
            np.concatenate([srcp_a[m], dstl_a[m]], axis=1))
        im["Pm_a"] = Pm_a[m].reshape(128, totT * 128)
        in_maps.append(im)

    return in_maps, Tb.tolist(), off.tolist(), totT


# ---------------------------------------------------------------------------
# device program
# ---------------------------------------------------------------------------

def build_program(cfg, Tb, off, totT):
    from concourse import bacc, bass, mybir, tile
    from concourse.masks import make_identity

    f32 = mybir.dt.float32
    bf16 = mybir.dt.bfloat16
    fp8 = mybir.dt.float8e4
    i32 = mybir.dt.int32
    X = mybir.AxisListType.X
    AF = mybir.ActivationFunctionType
    NC, NP, OWN_PAD, BLK, KC, H, C = (
        cfg["NC"], cfg["NP"], cfg["OWN_PAD"], cfg["BLK"], cfg["KC"],
        cfg["HEADS"], cfg["C"])
    HC = H * C
    W = 8 + HC               # table row width: [h (128) | a_src (8)]
    CB0, CH0, CH1 = cfg["CB0"], cfg["CH0"], cfg["CH1"]

    nc = bacc.Bacc("TRN2", target_bir_lowering=False, debug=False,
                   num_devices=NC)

    # inputs
    t_xTb = nc.dram_tensor("xTb", [128, KC, OWN_PAD], fp8,
                           kind="ExternalInput")
    t_W1b = nc.dram_tensor("W1b", [128, KC, HC], fp8, kind="ExternalInput")
    t_attS1 = nc.dram_tensor("attS1", [128, HC], bf16, kind="ExternalInput")
    t_attD1 = nc.dram_tensor("attD1", [128, HC], bf16, kind="ExternalInput")
    t_b1b = nc.dram_tensor("b1b", [128, HC], f32, kind="ExternalInput")
    t_W2 = nc.dram_tensor("W2", [HC, C], bf16, kind="ExternalInput")
    t_vavd = nc.dram_tensor("vavd", [HC, 2 * H], bf16, kind="ExternalInput")
    t_b2b = nc.dram_tensor("b2b", [128, C], f32, kind="ExternalInput")
    t_sd = nc.dram_tensor("sd_a", [128, 2 * totT], i32, kind="ExternalInput")
    t_Pm = nc.dram_tensor("Pm_a", [128, totT * 128], fp8,
                          kind="ExternalInput")
    t_out = nc.dram_tensor("out", [OWN_PAD, C], f32, kind="ExternalOutput")

    def fv(ap, dims, extra_offset=0):
        """View `ap` with custom free-dim [step, count] pairs."""
        return bass.AP(ap.tensor, ap.offset + extra_offset, [ap.ap[0]] + dims)

    with tile.TileContext(nc) as tc:
        with (
            tc.tile_pool(name="const", bufs=1) as cpool,
            tc.tile_pool(name="dram", bufs=1, space="DRAM") as dram,
        ):
            # ---------------- constants ----------------
            W1_sb = cpool.tile([128, KC * HC], fp8, tag="w1")
            nc.sync.dma_start(
                out=fv(W1_sb[:], [[HC, KC], [1, HC]]), in_=t_W1b[:, :, :])
            attS1_sb = cpool.tile([128, HC], bf16, tag="attS")
            nc.sync.dma_start(out=attS1_sb[:], in_=t_attS1[:, :])
            attD1_sb = cpool.tile([128, HC], bf16, tag="attD")
            nc.sync.dma_start(out=attD1_sb[:], in_=t_attD1[:, :])
            b1_sb = cpool.tile([128, HC], f32, tag="b1")
            nc.sync.dma_start(out=b1_sb[:], in_=t_b1b[:, :])
            W2_sb = cpool.tile([HC, C], bf16, tag="w2")
            nc.sync.dma_start(out=W2_sb[:], in_=t_W2[:, :])
            vavd_sb = cpool.tile([HC, 2 * H], bf16, tag="vavd")
            nc.sync.dma_start(out=vavd_sb[:], in_=t_vavd[:, :])
            b2_sb = cpool.tile([128, C], f32, tag="b2")
            nc.sync.dma_start(out=b2_sb[:], in_=t_b2b[:, :])
            identb = cpool.tile([128, 128], bf16, tag="identb")
            make_identity(nc, identb[:])
            # final logits staging for the batched log_softmax pass
            redall = cpool.tile([128, BLK * C], f32, tag="redall")

            # internal DRAM
            h1own = dram.tile([OWN_PAD, W], bf16, tag="h1own")
            adst1 = dram.tile([OWN_PAD, H], f32, tag="adst1")
            table1 = dram.tile([NP, W], bf16, tag="table1",
                               addr_space="Shared")
            h2own = dram.tile([OWN_PAD, W], bf16, tag="h2own")
            adst2 = dram.tile([OWN_PAD, H], f32, tag="adst2")
            table2 = dram.tile([NP, W], bf16, tag="table2",
                               addr_space="Shared")

            def allgather2(src_t, dst_t):
                # Shared-space DRAM only admits a single writing instruction,
                # so the gather cannot be chunked for overlap.
                nc.gpsimd.collective_compute(
                    "AllGather", mybir.AluOpType.bypass,
                    replica_groups=[list(range(NC))],
                    ins=[src_t[:].opt()],
                    outs=[dst_t[:].opt()])

            # ------- phase A: own-shard h1 table (h = x@W1, attn logits) ---
            # processed in groups of 4 128-row blocks to cut issue overhead
            with (
                tc.tile_pool(name="pa", bufs=4) as pa,
                tc.tile_pool(name="pa_ps", bufs=2, space="PSUM") as pa_ps,
            ):
                i0 = 0
                while i0 < BLK:
                    nb = min(4, BLK - i0)
                    r0 = i0 * 128
                    R = nb * 128
                    xt = pa.tile([128, KC * R], fp8, tag=f"xt{nb}")
                    nc.sync.dma_start(
                        out=fv(xt[:], [[R, KC], [1, R]]),
                        in_=t_xTb[:, :, r0:r0 + R])
                    hbp = pa.tile([128, nb * W], bf16, tag=f"hbp{nb}")
                    for h in range(nb):
                        ph = pa_ps.tile([128, HC], f32, tag=f"ph{h}")
                        for k in range(KC):
                            nc.tensor.matmul(
                                out=ph[:],
                                lhsT=xt[:, k * R + h * 128:
                                        k * R + (h + 1) * 128],
                                rhs=W1_sb[:, k * HC:(k + 1) * HC],
                                start=(k == 0), stop=(k == KC - 1))
                        nc.scalar.copy(out=hbp[:, h * W:h * W + HC], in_=ph[:])
                    # attention logits from the bf16 h (same as gathers see)
                    tmpS = pa.tile([128, nb * HC], bf16, tag=f"tmpS{nb}")
                    nc.vector.tensor_tensor(
                        out=fv(tmpS[:], [[HC, nb], [1, HC]]),
                        in0=fv(hbp[:], [[W, nb], [1, HC]]),
                        in1=fv(attS1_sb[:], [[0, nb], [1, HC]]),
                        op=mybir.AluOpType.mult)
                    asr = pa.tile([128, nb * H], f32, tag=f"asr{nb}")
                    nc.vector.reduce_sum(
                        out=asr[:], in_=fv(tmpS[:], [[C, nb * H], [1, C]]),
                        axis=X)
                    nc.scalar.copy(
                        out=fv(hbp[:], [[W, nb], [1, H]], extra_offset=HC),
                        in_=asr[:])
                    tmpD = pa.tile([128, nb * HC], bf16, tag=f"tmpD{nb}")
                    nc.vector.tensor_tensor(
                        out=fv(tmpD[:], [[HC, nb], [1, HC]]),
                        in0=fv(hbp[:], [[W, nb], [1, HC]]),
                        in1=fv(attD1_sb[:], [[0, nb], [1, HC]]),
                        op=mybir.AluOpType.mult)
                    adt = pa.tile([128, nb * H], f32, tag=f"adt{nb}")
                    nc.vector.reduce_sum(
                        out=adt[:], in_=fv(tmpD[:], [[C, nb * H], [1, C]]),
                        axis=X)
                    h1ap = h1own[:]
                    nc.sync.dma_start(
                        out=bass.AP(h1ap.tensor, h1ap.offset + r0 * W,
                                    [[W, 128], [128 * W, nb], [1, W]]),
                        in_=fv(hbp[:], [[W, nb], [1, W]]))
                    a1ap = adst1[:]
                    nc.sync.dma_start(
                        out=bass.AP(a1ap.tensor, a1ap.offset + r0 * H,
                                    [[H, 128], [128 * H, nb], [1, H]]),
                        in_=fv(adt[:], [[H, nb], [1, H]]))
                    i0 += nb

            allgather2(h1own, table1)

            # ---------------- edge phase (shared between layers) ----------
            # pairs of dst blocks per iteration: one gather / one one-hot /
            # one weighting op per pair, two PSUM scatter groups
            def edge_phase(tag, tab, adst_tab, finish):
                with (
                    tc.tile_pool(name=f"eg{tag}", bufs=4) as eg,
                    tc.tile_pool(name=f"epm{tag}", bufs=6) as epm,
                    tc.tile_pool(name=f"ew{tag}", bufs=3) as ew,
                    tc.tile_pool(name=f"ef{tag}", bufs=3) as ef,
                    tc.tile_pool(name=f"eps{tag}", bufs=2, space="PSUM") as eps,
                    tc.tile_pool(name=f"fps{tag}", bufs=2, space="PSUM") as fps,
                ):
                    for ip in range(BLK // 2):
                        b0 = 2 * ip
                        T0, T1 = Tb[b0], Tb[b0 + 1]
                        TP = T0 + T1
                        o = off[b0]
                        idxd = eg.tile([128, 2 * TP], i32, tag="idxd")
                        nc.sync.dma_start(
                            out=idxd[:],
                            in_=fv(t_sd[:, 0:1], [[totT, 2], [1, TP]],
                                   extra_offset=o))

                        gath = eg.tile([128, TP * W], bf16, tag="gath")
                        nc.gpsimd.indirect_dma_start(
                            out=gath[:], out_offset=None,
                            in_=tab[:, :],
                            in_offset=bass.IndirectOffsetOnAxis(
                                ap=idxd[:, 0:TP], axis=0))
                        gd = eg.tile([128, TP * H], f32, tag="gd")
                        nc.gpsimd.indirect_dma_start(
                            out=gd[:], out_offset=None,
                            in_=adst_tab[:, :],
                            in_offset=bass.IndirectOffsetOnAxis(
                                ap=idxd[:, TP:2 * TP], axis=0))

                        # static one-hot Pm[e, (j, d)] streamed from DRAM
                        Pm = epm.tile([128, TP * 128], fp8, tag="Pm")
                        nc.sync.dma_start(
                            out=Pm[:], in_=t_Pm[:, o * 128:(o + TP) * 128])

                        # alpha = a_src[src] + a_dst[dst]; ex = exp(lrelu(.))
                        ax = ef.tile([128, TP * H], f32, tag="ax")
                        nc.vector.tensor_add(
                            out=ax[:],
                            in0=fv(gath[:], [[W, TP], [1, H]],
                                   extra_offset=HC),
                            in1=gd[:])
                        nc.vector.scalar_tensor_tensor(
                            out=ax[:], in0=ax[:], scalar=NEG_SLOPE,
                            in1=ax[:], op0=mybir.AluOpType.mult,
                            op1=mybir.AluOpType.max)
                        ex = ef.tile([128, TP * H], f32, tag="ex")
                        nc.scalar.activation(out=ex[:], in_=ax[:], func=AF.Exp)

                        # weighted features (+ ex appended per tile): [T, 136]
                        wt = ew.tile([128, TP * W], bf16, tag="wt")
                        nc.vector.tensor_tensor(
                            out=fv(wt[:], [[W, TP], [C, H], [1, C]]),
                            in0=fv(gath[:], [[W, TP], [C, H], [1, C]]),
                            in1=fv(ex[:], [[H, TP], [1, H], [0, C]]),
                            op=mybir.AluOpType.mult)
                        nc.scalar.copy(
                            out=fv(wt[:], [[W, TP], [1, H]], extra_offset=HC),
                            in_=ex[:])

                        for h, (lo, hi) in enumerate(((0, T0), (T0, TP))):
                            nd = eps.tile([128, W], f32, tag=f"nd{h}")
                            for j in range(lo, hi):
                                nc.tensor.matmul(
                                    out=nd[:],
                                    lhsT=Pm[:, j * 128:(j + 1) * 128],
                                    rhs=wt[:, j * W:(j + 1) * W],
                                    start=(j == lo), stop=(j == hi - 1))
                            finish(b0 + h, nd, ef, fps)

            # ---------------- layer-1 block finisher ----------------------
            def finish1(b, nd, ef, fps):
                dr = ef.tile([128, H], f32, tag="dr")
                nc.vector.tensor_scalar_add(dr[:], nd[:, HC:W], EPS)
                nc.vector.reciprocal(out=dr[:], in_=dr[:])
                g = ef.tile([128, HC], f32, tag="g")
                nc.vector.tensor_tensor(
                    out=fv(g[:], [[C, H], [1, C]]),
                    in0=fv(nd[:], [[C, H], [1, C]]),
                    in1=fv(dr[:], [[1, H], [0, C]]),
                    op=mybir.AluOpType.mult)
                nc.vector.tensor_add(out=g[:], in0=g[:], in1=b1_sb[:])
                # ELU
                tn = ef.tile([128, HC], f32, tag="tn")
                nc.vector.tensor_scalar_min(tn[:], g[:], 0.0)
                te = ef.tile([128, HC], f32, tag="te")
                nc.scalar.activation(out=te[:], in_=tn[:], func=AF.Exp)
                nc.vector.scalar_tensor_tensor(
                    out=g[:], in0=g[:], scalar=0.0, in1=te[:],
                    op0=mybir.AluOpType.max, op1=mybir.AluOpType.add)
                # pack h2 row [ELU | a_src2] (bf16); the ELU "-1" rides the
                # cast's bias so it costs no extra op
                hb2 = ef.tile([128, W], bf16, tag="hb2")
                nc.scalar.activation(out=hb2[:, :HC], in_=g[:], func=AF.Copy,
                                     bias=-1.0)
                # a_src2 / a_dst2 via g @ (W2 @ att2): needs gT as lhsT
                gtp = fps.tile([128, 128], bf16, tag="gtp")
                nc.tensor.transpose(out=gtp[:], in_=hb2[:, :HC],
                                    identity=identb[:])
                gts = ef.tile([128, 128], bf16, tag="gts")
                nc.scalar.copy(out=gts[:], in_=gtp[:])
                a2p = fps.tile([128, 2 * H], f32, tag="a2p")
                nc.tensor.matmul(out=a2p[:], lhsT=gts[:], rhs=vavd_sb[:],
                                 start=True, stop=True)
                a2s = ef.tile([128, 2 * H], f32, tag="a2s")
                nc.scalar.copy(out=a2s[:], in_=a2p[:])
                nc.scalar.copy(out=hb2[:, HC:W], in_=a2s[:, :H])
                nc.sync.dma_start(
                    out=h2own[b * 128:(b + 1) * 128, :], in_=hb2[:])
                nc.sync.dma_start(
                    out=adst2[b * 128:(b + 1) * 128, :], in_=a2s[:, H:])

            # ---------------- layer-2 block finisher ----------------------
            # (log_softmax deferred to one batched pass -> no per-block
            #  activation-table thrash between EXP and LN)
            def finish2(b, nd, ef, fps):
                dr = ef.tile([128, H], f32, tag="dr")
                nc.vector.tensor_scalar_add(dr[:], nd[:, HC:W], EPS)
                nc.vector.reciprocal(out=dr[:], in_=dr[:])
                g = ef.tile([128, HC], bf16, tag="g")
                nc.vector.tensor_tensor(
                    out=fv(g[:], [[C, H], [1, C]]),
                    in0=fv(nd[:], [[C, H], [1, C]]),
                    in1=fv(dr[:], [[1, H], [0, C]]),
                    op=mybir.AluOpType.mult)
                atp = fps.tile([128, 128], bf16, tag="gtp")
                nc.tensor.transpose(out=atp[:], in_=g[:], identity=identb[:])
                ats = ef.tile([128, 128], bf16, tag="gts")
                nc.scalar.copy(out=ats[:], in_=atp[:])
                o2 = fps.tile([128, C], f32, tag="o2")
                nc.tensor.matmul(out=o2[:], lhsT=ats[:], rhs=W2_sb[:],
                                 start=True, stop=True)
                nc.scalar.copy(out=redall[:, b * C:(b + 1) * C], in_=o2[:])

            # ---------------- run both layers ------------------------------
            edge_phase("1", table1, adst1, finish1)

            allgather2(h2own, table2)

            edge_phase("2", table2, adst2, finish2)

            # ---------------- batched bias + log_softmax ------------------
            with tc.tile_pool(name="fin", bufs=1) as fin:
                nc.vector.tensor_add(
                    out=fv(redall[:], [[C, BLK], [1, C]]),
                    in0=fv(redall[:], [[C, BLK], [1, C]]),
                    in1=fv(b2_sb[:], [[0, BLK], [1, C]]))
                mx = fin.tile([128, BLK], f32, tag="mx")
                nc.vector.reduce_max(
                    out=mx[:], in_=fv(redall[:], [[C, BLK], [1, C]]), axis=X)
                nc.vector.tensor_tensor(
                    out=fv(redall[:], [[C, BLK], [1, C]]),
                    in0=fv(redall[:], [[C, BLK], [1, C]]),
                    in1=fv(mx[:], [[1, BLK], [0, C]]),
                    op=mybir.AluOpType.subtract)
                es = fin.tile([128, BLK * C], f32, tag="es")
                nc.scalar.activation(out=es[:], in_=redall[:], func=AF.Exp)
                sm = fin.tile([128, BLK], f32, tag="sm")
                nc.vector.reduce_sum(
                    out=sm[:], in_=fv(es[:], [[C, BLK], [1, C]]), axis=X)
                ls = fin.tile([128, BLK], f32, tag="ls")
                nc.scalar.activation(out=ls[:], in_=sm[:], func=AF.Ln)
                nc.vector.tensor_tensor(
                    out=fv(redall[:], [[C, BLK], [1, C]]),
                    in0=fv(redall[:], [[C, BLK], [1, C]]),
                    in1=fv(ls[:], [[1, BLK], [0, C]]),
                    op=mybir.AluOpType.subtract)
                oap = t_out[:, :]
                nc.sync.dma_start(
                    out=bass.AP(oap.tensor, oap.offset,
                                [[C, 128], [128 * C, BLK], [1, C]]),
                    in_=redall[:])

    nc.compile()
    return nc


# ---------------------------------------------------------------------------
# entry point
# ---------------------------------------------------------------------------

def _run(cfg, inputs, trace=False):
    from concourse.bass_utils import run_bass_kernel_spmd

    cfg = _derive(cfg)
    in_maps, Tb, off, totT = _host_prep(cfg, **inputs)
    nc = build_program(cfg, Tb, off, totT)
    res = run_bass_kernel_spmd(
        nc, in_maps, core_ids=list(range(cfg["NC"])), trace=trace)
    outs = []
    for m in range(cfg["NC"]):
        outs.append(res.results[m]["out"][:cfg["OWN"]])
    full = np.concatenate(outs, axis=0)
    return full, res


def kernel(x, edge_index, W1, att_src1, att_dst1, bias1, W2, att_src2,
           att_dst2, bias2):
    inputs = dict(x=np.asarray(x, np.float32),
                  edge_index=np.asarray(edge_index),
                  W1=W1, att_src1=att_src1, att_dst1=att_dst1, bias1=bias1,
                  W2=W2, att_src2=att_src2, att_dst2=att_dst2, bias2=bias2)
    out, _ = _run(FULL_CFG, inputs, trace=False)
    return out


# revision 31
# speedup vs baseline: 1.2601x; 1.0222x over previous
"""Two-layer GAT on 8 Trainium2 NeuronCores (Bass/Tile SPMD kernel).

Sharding: nodes are range-partitioned across the 8 cores; each core owns the
edges whose *destination* falls in its partition (segment softmax is per-dst,
so the softmax/aggregation is fully core-local). Each layer's node feature
table ([h | a_src], bf16) is computed sharded (each core does its own nodes)
and AllGathered in two chunks (so the first chunk's transfer overlaps the
producer's tail); the a_dst table stays core-local since every edge's dst is
owned by the core that processes it.

Per-edge math uses the shift-invariance of softmax:
    out_d = sum_e exp(alpha_e) * h[src_e] / sum_e exp(alpha_e)
so a single pass over the edges suffices (no segment max needed; alpha is
O(1) here so exp cannot overflow).
"""

import sys

sys.path.insert(0, "/opt/trn_rl_repo")

import numpy as np

try:
    from ml_dtypes import bfloat16 as np_bf16
    from ml_dtypes import float8_e4m3 as np_fp8
except ImportError:
    import jax.numpy as _jnp
    np_bf16 = _jnp.bfloat16
    np_fp8 = _jnp.float8_e4m3

# ---------------------------------------------------------------------------
# configuration
# ---------------------------------------------------------------------------

FULL_CFG = dict(
    N=100000,      # real nodes
    IN_CH=512,     # input features
    HEADS=8,
    C=16,          # out channels per head
    NC=8,          # cores
)

NEG_SLOPE = 0.2
EPS = 1e-16


def _derive(cfg):
    d = dict(cfg)
    d["HC"] = d["HEADS"] * d["C"]                 # 128
    assert d["HC"] == 128
    assert d["IN_CH"] % 128 == 0
    d["KC"] = d["IN_CH"] // 128                   # k-chunks for x@W1
    assert d["N"] % d["NC"] == 0
    d["OWN"] = d["N"] // d["NC"]                  # real nodes per core
    d["BLK"] = (d["OWN"] + 127) // 128            # dst blocks per core
    assert d["BLK"] % 2 == 0
    d["OWN_PAD"] = d["BLK"] * 128
    d["NP"] = d["NC"] * d["OWN_PAD"]              # padded global nodes
    # AllGather chunking: chunk 0 = first CB0 blocks, chunk 1 = the rest
    d["CB0"] = (d["BLK"] // 2 + 1) // 2 * 2       # 50 blocks (pair-aligned)
    d["CH0"] = d["CB0"] * 128                     # rows in chunk 0
    d["CH1"] = d["OWN_PAD"] - d["CH0"]
    return d


# ---------------------------------------------------------------------------
# host-side prep: edge partitioning / padding / layouts
# ---------------------------------------------------------------------------

def _host_prep(cfg, x, edge_index, W1, att_src1, att_dst1, bias1, W2,
               att_src2, att_dst2, bias2):
    N, NC, OWN, BLK, OWN_PAD, NP, KC, H, C = (
        cfg["N"], cfg["NC"], cfg["OWN"], cfg["BLK"], cfg["OWN_PAD"],
        cfg["NP"], cfg["KC"], cfg["HEADS"], cfg["C"])
    HC = H * C
    CH0, CH1 = cfg["CH0"], cfg["CH1"]

    src = np.asarray(edge_index[0], dtype=np.int64)
    dst = np.asarray(edge_index[1], dtype=np.int64)

    core = dst // OWN                       # owning core of each edge
    ldst = (dst - core * OWN).astype(np.int64)    # local dst id [0, OWN)
    srcp = ((src // OWN) * OWN_PAD + (src % OWN)).astype(np.int32)  # padded gid
    blk = ldst // 128
    slot = ldst % 128

    # per-(core, block) edge counts -> shared tile counts T_b (SPMD uniform)
    counts = np.zeros((NC, BLK), dtype=np.int64)
    np.add.at(counts, (core, blk), 1)
    Tb = np.maximum(1, (counts.max(axis=0) + 127) // 128).astype(np.int64)
    off = np.zeros(BLK, dtype=np.int64)
    off[1:] = np.cumsum(Tb)[:-1]
    totT = int(Tb.sum())

    # per-core edge arrays, laid out [128, totT]:
    #   edge e of block b  ->  (partition p = e%128, column off[b] + e//128)
    srcp_a = np.zeros((NC, 128, totT), dtype=np.int32)
    dstl_a = np.zeros((NC, 128, totT), dtype=np.int32)
    # static one-hot scatter matrices Pm[p, col, d] = (slot == d), bf16
    Pm_a = np.zeros((NC, 128, totT, 128), dtype=np_fp8)

    # sort edges within each (core, block) group by source row so the
    # indirect gather walks the table nearly sequentially (HBM page hits)
    order = np.lexsort((srcp, blk, core))
    s_core, s_blk = core[order], blk[order]
    s_srcp, s_ldst, s_slot = srcp[order], ldst[order], slot[order]
    grp = s_core * BLK + s_blk
    first = np.ones(len(grp), dtype=bool)
    first[1:] = grp[1:] != grp[:-1]
    starts = np.flatnonzero(first)
    group_start = np.repeat(starts, np.diff(np.append(starts, len(grp))))
    pos = np.arange(len(grp)) - group_start
    pp = pos % 128
    col = off[s_blk] + pos // 128
    srcp_a[s_core, pp, col] = s_srcp
    dstl_a[s_core, pp, col] = s_ldst
    Pm_a[s_core, pp, col, s_slot] = np_fp8(1.0)

    xf = np.asarray(x, np.float32)
    attS1 = np.broadcast_to(
        np.asarray(att_src1, np.float32).reshape(1, HC).astype(np_bf16),
        (128, HC)).copy()
    attD1 = np.broadcast_to(
        np.asarray(att_dst1, np.float32).reshape(1, HC).astype(np_bf16),
        (128, HC)).copy()
    b1b = np.broadcast_to(
        np.asarray(bias1, np.float32).reshape(1, HC), (128, HC)).copy()
    W1b = np.ascontiguousarray(
        np.asarray(W1, np.float32).reshape(KC, 128, HC).transpose(1, 0, 2)
    ).astype(np_fp8)
    W2f = np.asarray(W2, np.float32)
    va = np.einsum("khc,hc->kh", W2f.reshape(HC, H, C),
                   np.asarray(att_src2, np.float32))
    vd = np.einsum("khc,hc->kh", W2f.reshape(HC, H, C),
                   np.asarray(att_dst2, np.float32))
    vavd = np.ascontiguousarray(
        np.concatenate([va, vd], axis=1)).astype(np_bf16)  # [128, 2H]
    b2b = np.broadcast_to(
        np.asarray(bias2, np.float32).reshape(1, C), (128, C)).copy()

    W2s = W2f.reshape(HC, H, C).mean(axis=1)          # head-mean folded in
    shared = dict(W1b=W1b, attS1=attS1, attD1=attD1,
                  b1b=b1b, W2=np.ascontiguousarray(W2s).astype(np_bf16),
                  vavd=vavd, b2b=b2b)
    in_maps = []
    for m in range(NC):
        im = dict(shared)
        xp = np.zeros((OWN_PAD, cfg["IN_CH"]), dtype=np.float32)
        xp[:OWN] = xf[m * OWN:(m + 1) * OWN]
        im["xTb"] = np.ascontiguousarray(
            xp.reshape(OWN_PAD, KC, 128).transpose(2, 1, 0)).astype(np_fp8)
        im["sd_a"] = np.ascontiguousarray(
            np.concatenate([srcp_a[m], dstl_a[m]], axis=1))
        im["Pm_a"] = Pm_a[m].reshape(128, totT * 128)
        in_maps.append(im)

    return in_maps, Tb.tolist(), off.tolist(), totT


# ---------------------------------------------------------------------------
# device program
# ---------------------------------------------------------------------------

def build_program(cfg, Tb, off, totT):
    from concourse import bacc, bass, mybir, tile
    from concourse.masks import make_identity

    f32 = mybir.dt.float32
    bf16 = mybir.dt.bfloat16
    fp8 = mybir.dt.float8e4
    i32 = mybir.dt.int32
    X = mybir.AxisListType.X
    AF = mybir.ActivationFunctionType
    NC, NP, OWN_PAD, BLK, KC, H, C = (
        cfg["NC"], cfg["NP"], cfg["OWN_PAD"], cfg["BLK"], cfg["KC"],
        cfg["HEADS"], cfg["C"])
    HC = H * C
    W = 8 + HC               # table row width: [h (128) | a_src (8)]
    CB0, CH0, CH1 = cfg["CB0"], cfg["CH0"], cfg["CH1"]

    nc = bacc.Bacc("TRN2", target_bir_lowering=False, debug=False,
                   num_devices=NC)

    # inputs
    t_xTb = nc.dram_tensor("xTb", [128, KC, OWN_PAD], fp8,
                           kind="ExternalInput")
    t_W1b = nc.dram_tensor("W1b", [128, KC, HC], fp8, kind="ExternalInput")
    t_attS1 = nc.dram_tensor("attS1", [128, HC], bf16, kind="ExternalInput")
    t_attD1 = nc.dram_tensor("attD1", [128, HC], bf16, kind="ExternalInput")
    t_b1b = nc.dram_tensor("b1b", [128, HC], f32, kind="ExternalInput")
    t_W2 = nc.dram_tensor("W2", [HC, C], bf16, kind="ExternalInput")
    t_vavd = nc.dram_tensor("vavd", [HC, 2 * H], bf16, kind="ExternalInput")
    t_b2b = nc.dram_tensor("b2b", [128, C], f32, kind="ExternalInput")
    t_sd = nc.dram_tensor("sd_a", [128, 2 * totT], i32, kind="ExternalInput")
    t_Pm = nc.dram_tensor("Pm_a", [128, totT * 128], fp8,
                          kind="ExternalInput")
    t_out = nc.dram_tensor("out", [OWN_PAD, C], f32, kind="ExternalOutput")

    def fv(ap, dims, extra_offset=0):
        """View `ap` with custom free-dim [step, count] pairs."""
        return bass.AP(ap.tensor, ap.offset + extra_offset, [ap.ap[0]] + dims)

    with tile.TileContext(nc) as tc:
        with (
            tc.tile_pool(name="const", bufs=1) as cpool,
            tc.tile_pool(name="dram", bufs=1, space="DRAM") as dram,
        ):
            # ---------------- constants ----------------
            W1_sb = cpool.tile([128, KC * HC], fp8, tag="w1")
            nc.sync.dma_start(
                out=fv(W1_sb[:], [[HC, KC], [1, HC]]), in_=t_W1b[:, :, :])
            attS1_sb = cpool.tile([128, HC], bf16, tag="attS")
            nc.sync.dma_start(out=attS1_sb[:], in_=t_attS1[:, :])
            attD1_sb = cpool.tile([128, HC], bf16, tag="attD")
            nc.sync.dma_start(out=attD1_sb[:], in_=t_attD1[:, :])
            b1_sb = cpool.tile([128, HC], f32, tag="b1")
            nc.sync.dma_start(out=b1_sb[:], in_=t_b1b[:, :])
            W2_sb = cpool.tile([HC, C], bf16, tag="w2")
            nc.sync.dma_start(out=W2_sb[:], in_=t_W2[:, :])
            vavd_sb = cpool.tile([HC, 2 * H], bf16, tag="vavd")
            nc.sync.dma_start(out=vavd_sb[:], in_=t_vavd[:, :])
            b2_sb = cpool.tile([128, C], f32, tag="b2")
            nc.sync.dma_start(out=b2_sb[:], in_=t_b2b[:, :])
            identb = cpool.tile([128, 128], bf16, tag="identb")
            make_identity(nc, identb[:])
            # final logits staging for the batched log_softmax pass
            redall = cpool.tile([128, BLK * C], f32, tag="redall")

            # internal DRAM
            h1own = dram.tile([OWN_PAD, W], bf16, tag="h1own")
            adst1 = dram.tile([OWN_PAD, H], f32, tag="adst1")
            table1 = dram.tile([NP, W], bf16, tag="table1",
                               addr_space="Shared")
            h2own = dram.tile([OWN_PAD, W], bf16, tag="h2own")
            adst2 = dram.tile([OWN_PAD, H], f32, tag="adst2")
            table2 = dram.tile([NP, W], bf16, tag="table2",
                               addr_space="Shared")

            def allgather2(src_t, dst_t):
                # Shared-space DRAM only admits a single writing instruction,
                # so the gather cannot be chunked for overlap.
                nc.gpsimd.collective_compute(
                    "AllGather", mybir.AluOpType.bypass,
                    replica_groups=[list(range(NC))],
                    ins=[src_t[:].opt()],
                    outs=[dst_t[:].opt()])

            # ------- phase A: own-shard h1 table (h = x@W1, attn logits) ---
            # processed in groups of 4 128-row blocks to cut issue overhead
            with (
                tc.tile_pool(name="pa", bufs=4) as pa,
                tc.tile_pool(name="pa_ps", bufs=2, space="PSUM") as pa_ps,
            ):
                i0 = 0
                while i0 < BLK:
                    nb = min(4, BLK - i0)
                    r0 = i0 * 128
                    R = nb * 128
                    xt = pa.tile([128, KC * R], fp8, tag=f"xt{nb}")
                    nc.sync.dma_start(
                        out=fv(xt[:], [[R, KC], [1, R]]),
                        in_=t_xTb[:, :, r0:r0 + R])
                    hbp = pa.tile([128, nb * W], bf16, tag=f"hbp{nb}")
                    for h in range(nb):
                        ph = pa_ps.tile([128, HC], f32, tag=f"ph{h}")
                        for k in range(KC):
                            nc.tensor.matmul(
                                out=ph[:],
                                lhsT=xt[:, k * R + h * 128:
                                        k * R + (h + 1) * 128],
                                rhs=W1_sb[:, k * HC:(k + 1) * HC],
                                start=(k == 0), stop=(k == KC - 1))
                        nc.scalar.copy(out=hbp[:, h * W:h * W + HC], in_=ph[:])
                    # attention logits from the bf16 h (same as gathers see)
                    tmpS = pa.tile([128, nb * HC], bf16, tag=f"tmpS{nb}")
                    nc.vector.tensor_tensor(
                        out=fv(tmpS[:], [[HC, nb], [1, HC]]),
                        in0=fv(hbp[:], [[W, nb], [1, HC]]),
                        in1=fv(attS1_sb[:], [[0, nb], [1, HC]]),
                        op=mybir.AluOpType.mult)
                    asr = pa.tile([128, nb * H], f32, tag=f"asr{nb}")
                    nc.vector.reduce_sum(
                        out=asr[:], in_=fv(tmpS[:], [[C, nb * H], [1, C]]),
                        axis=X)
                    nc.scalar.copy(
                        out=fv(hbp[:], [[W, nb], [1, H]], extra_offset=HC),
                        in_=asr[:])
                    tmpD = pa.tile([128, nb * HC], bf16, tag=f"tmpD{nb}")
                    nc.vector.tensor_tensor(
                        out=fv(tmpD[:], [[HC, nb], [1, HC]]),
                        in0=fv(hbp[:], [[W, nb], [1, HC]]),
                        in1=fv(attD1_sb[:], [[0, nb], [1, HC]]),
                        op=mybir.AluOpType.mult)
                    adt = pa.tile([128, nb * H], f32, tag=f"adt{nb}")
                    nc.vector.reduce_sum(
                        out=adt[:], in_=fv(tmpD[:], [[C, nb * H], [1, C]]),
                        axis=X)
                    h1ap = h1own[:]
                    nc.sync.dma_start(
                        out=bass.AP(h1ap.tensor, h1ap.offset + r0 * W,
                                    [[W, 128], [128 * W, nb], [1, W]]),
                        in_=fv(hbp[:], [[W, nb], [1, W]]))
                    a1ap = adst1[:]
                    nc.sync.dma_start(
                        out=bass.AP(a1ap.tensor, a1ap.offset + r0 * H,
                                    [[H, 128], [128 * H, nb], [1, H]]),
                        in_=fv(adt[:], [[H, nb], [1, H]]))
                    i0 += nb

            allgather2(h1own, table1)

            # ---------------- edge phase (shared between layers) ----------
            # pairs of dst blocks per iteration: one gather / one one-hot /
            # one weighting op per pair, two PSUM scatter groups
            def edge_phase(tag, tab, adst_tab, finish):
                with (
                    tc.tile_pool(name=f"eg{tag}", bufs=4) as eg,
                    tc.tile_pool(name=f"epm{tag}", bufs=6) as epm,
                    tc.tile_pool(name=f"ew{tag}", bufs=3) as ew,
                    tc.tile_pool(name=f"ef{tag}", bufs=3) as ef,
                    tc.tile_pool(name=f"eps{tag}", bufs=2, space="PSUM") as eps,
                    tc.tile_pool(name=f"fps{tag}", bufs=2, space="PSUM") as fps,
                ):
                    for ip in range(BLK // 2):
                        b0 = 2 * ip
                        T0, T1 = Tb[b0], Tb[b0 + 1]
                        TP = T0 + T1
                        o = off[b0]
                        idxd = eg.tile([128, 2 * TP], i32, tag="idxd")
                        nc.sync.dma_start(
                            out=idxd[:],
                            in_=fv(t_sd[:, 0:1], [[totT, 2], [1, TP]],
                                   extra_offset=o))

                        gath = eg.tile([128, TP * W], bf16, tag="gath")
                        nc.gpsimd.indirect_dma_start(
                            out=gath[:], out_offset=None,
                            in_=tab[:, :],
                            in_offset=bass.IndirectOffsetOnAxis(
                                ap=idxd[:, 0:TP], axis=0))
                        gd = eg.tile([128, TP * H], f32, tag="gd")
                        nc.gpsimd.indirect_dma_start(
                            out=gd[:], out_offset=None,
                            in_=adst_tab[:, :],
                            in_offset=bass.IndirectOffsetOnAxis(
                                ap=idxd[:, TP:2 * TP], axis=0))

                        # static one-hot Pm[e, (j, d)] streamed from DRAM
                        Pm = epm.tile([128, TP * 128], fp8, tag="Pm")
                        nc.sync.dma_start(
                            out=Pm[:], in_=t_Pm[:, o * 128:(o + TP) * 128])

                        # alpha = a_src[src] + a_dst[dst]; ex = exp(lrelu(.))
                        ax = ef.tile([128, TP * H], f32, tag="ax")
                        nc.vector.tensor_add(
                            out=ax[:],
                            in0=fv(gath[:], [[W, TP], [1, H]],
                                   extra_offset=HC),
                            in1=gd[:])
                        nc.vector.scalar_tensor_tensor(
                            out=ax[:], in0=ax[:], scalar=NEG_SLOPE,
                            in1=ax[:], op0=mybir.AluOpType.mult,
                            op1=mybir.AluOpType.max)
                        ex = ef.tile([128, TP * H], f32, tag="ex")
                        nc.scalar.activation(out=ex[:], in_=ax[:], func=AF.Exp)

                        # weighted features (+ ex appended per tile): [T, 136]
                        wt = ew.tile([128, TP * W], bf16, tag="wt")
                        nc.vector.tensor_tensor(
                            out=fv(wt[:], [[W, TP], [C, H], [1, C]]),
                            in0=fv(gath[:], [[W, TP], [C, H], [1, C]]),
                            in1=fv(ex[:], [[H, TP], [1, H], [0, C]]),
                            op=mybir.AluOpType.mult)
                        nc.scalar.copy(
                            out=fv(wt[:], [[W, TP], [1, H]], extra_offset=HC),
                            in_=ex[:])

                        for h, (lo, hi) in enumerate(((0, T0), (T0, TP))):
                            nd = eps.tile([128, W], f32, tag=f"nd{h}")
                            for j in range(lo, hi):
                                nc.tensor.matmul(
                                    out=nd[:],
                                    lhsT=Pm[:, j * 128:(j + 1) * 128],
                                    rhs=wt[:, j * W:(j + 1) * W],
                                    start=(j == lo), stop=(j == hi - 1))
                            finish(b0 + h, nd, ef, fps)

            # ---------------- layer-1 block finisher ----------------------
            def finish1(b, nd, ef, fps):
                dr = ef.tile([128, H], f32, tag="dr")
                nc.vector.tensor_scalar_add(dr[:], nd[:, HC:W], EPS)
                nc.vector.reciprocal(out=dr[:], in_=dr[:])
                g = ef.tile([128, HC], f32, tag="g")
                nc.vector.tensor_tensor(
                    out=fv(g[:], [[C, H], [1, C]]),
                    in0=fv(nd[:], [[C, H], [1, C]]),
                    in1=fv(dr[:], [[1, H], [0, C]]),
                    op=mybir.AluOpType.mult)
                nc.vector.tensor_add(out=g[:], in0=g[:], in1=b1_sb[:])
                # ELU
                tn = ef.tile([128, HC], f32, tag="tn")
                nc.vector.tensor_scalar_min(tn[:], g[:], 0.0)
                te = ef.tile([128, HC], f32, tag="te")
                nc.scalar.activation(out=te[:], in_=tn[:], func=AF.Exp)
                nc.vector.scalar_tensor_tensor(
                    out=g[:], in0=g[:], scalar=0.0, in1=te[:],
                    op0=mybir.AluOpType.max, op1=mybir.AluOpType.add)
                # pack h2 row [ELU | a_src2] (bf16); the ELU "-1" rides the
                # cast's bias so it costs no extra op
                hb2 = ef.tile([128, W], bf16, tag="hb2")
                nc.scalar.activation(out=hb2[:, :HC], in_=g[:], func=AF.Copy,
                                     bias=-1.0)
                # a_src2 / a_dst2 via g @ (W2 @ att2): needs gT as lhsT
                gtp = fps.tile([128, 128], bf16, tag="gtp")
                nc.tensor.transpose(out=gtp[:], in_=hb2[:, :HC],
                                    identity=identb[:])
                gts = ef.tile([128, 128], bf16, tag="gts")
                nc.scalar.copy(out=gts[:], in_=gtp[:])
                a2p = fps.tile([128, 2 * H], f32, tag="a2p")
                nc.tensor.matmul(out=a2p[:], lhsT=gts[:], rhs=vavd_sb[:],
                                 start=True, stop=True)
                a2s = ef.tile([128, 2 * H], f32, tag="a2s")
                nc.scalar.copy(out=a2s[:], in_=a2p[:])
                nc.scalar.copy(out=hb2[:, HC:W], in_=a2s[:, :H])
                nc.sync.dma_start(
                    out=h2own[b * 128:(b + 1) * 128, :], in_=hb2[:])
                nc.sync.dma_start(
                    out=adst2[b * 128:(b + 1) * 128, :], in_=a2s[:, H:])

            # ---------------- layer-2 block finisher ----------------------
            # (log_softmax deferred to one batched pass -> no per-block
            #  activation-table thrash between EXP and LN)
            def finish2(b, nd, ef, fps):
                dr = ef.tile([128, H], f32, tag="dr")
                nc.vector.tensor_scalar_add(dr[:], nd[:, HC:W], EPS)
                nc.vector.reciprocal(out=dr[:], in_=dr[:])
                g = ef.tile([128, HC], bf16, tag="g")
                nc.vector.tensor_tensor(
                    out=fv(g[:], [[C, H], [1, C]]),
                    in0=fv(nd[:], [[C, H], [1, C]]),
                    in1=fv(dr[:], [[1, H], [0, C]]),
                    op=mybir.AluOpType.mult)
                atp = fps.tile([128, 128], bf16, tag="gtp")
                nc.tensor.transpose(out=atp[:], in_=g[:], identity=identb[:])
                ats = ef.tile([128, 128], bf16, tag="gts")
                nc.scalar.copy(out=ats[:], in_=atp[:])
                o2 = fps.tile([128, C], f32, tag="o2")
                nc.tensor.matmul(out=o2[:], lhsT=ats[:], rhs=W2_sb[:],
                                 start=True, stop=True)
                nc.scalar.copy(out=redall[:, b * C:(b + 1) * C], in_=o2[:])

            # ---------------- run both layers ------------------------------
            edge_phase("1", table1, adst1, finish1)

            allgather2(h2own, table2)

            edge_phase("2", table2, adst2, finish2)

            # ---------------- batched bias + log_softmax ------------------
            with tc.tile_pool(name="fin", bufs=1) as fin:
                nc.vector.tensor_add(
                    out=fv(redall[:], [[C, BLK], [1, C]]),
                    in0=fv(redall[:], [[C, BLK], [1, C]]),
                    in1=fv(b2_sb[:], [[0, BLK], [1, C]]))
                mx = fin.tile([128, BLK], f32, tag="mx")
                nc.vector.reduce_max(
                    out=mx[:], in_=fv(redall[:], [[C, BLK], [1, C]]), axis=X)
                nc.vector.tensor_tensor(
                    out=fv(redall[:], [[C, BLK], [1, C]]),
                    in0=fv(redall[:], [[C, BLK], [1, C]]),
                    in1=fv(mx[:], [[1, BLK], [0, C]]),
                    op=mybir.AluOpType.subtract)
                es = fin.tile([128, BLK * C], f32, tag="es")
                nc.scalar.activation(out=es[:], in_=redall[:], func=AF.Exp)
                sm = fin.tile([128, BLK], f32, tag="sm")
                nc.vector.reduce_sum(
                    out=sm[:], in_=fv(es[:], [[C, BLK], [1, C]]), axis=X)
                ls = fin.tile([128, BLK], f32, tag="ls")
                nc.scalar.activation(out=ls[:], in_=sm[:], func=AF.Ln)
                nc.vector.tensor_tensor(
                    out=fv(redall[:], [[C, BLK], [1, C]]),
                    in0=fv(redall[:], [[C, BLK], [1, C]]),
                    in1=fv(ls[:], [[1, BLK], [0, C]]),
                    op=mybir.AluOpType.subtract)
                oap = t_out[:, :]
                nc.sync.dma_start(
                    out=bass.AP(oap.tensor, oap.offset,
                                [[C, 128], [128 * C, BLK], [1, C]]),
                    in_=redall[:])

    nc.compile()
    return nc


# ---------------------------------------------------------------------------
# entry point
# ---------------------------------------------------------------------------

def _run(cfg, inputs, trace=False):
    from concourse.bass_utils import run_bass_kernel_spmd

    cfg = _derive(cfg)
    in_maps, Tb, off, totT = _host_prep(cfg, **inputs)
    nc = build_program(cfg, Tb, off, totT)
    res = run_bass_kernel_spmd(
        nc, in_maps, core_ids=list(range(cfg["NC"])), trace=trace)
    outs = []
    for m in range(cfg["NC"]):
        outs.append(res.results[m]["out"][:cfg["OWN"]])
    full = np.concatenate(outs, axis=0)
    return full, res


def kernel(x, edge_index, W1, att_src1, att_dst1, bias1, W2, att_src2,
           att_dst2, bias2):
    inputs = dict(x=np.asarray(x, np.float32),
                  edge_index=np.asarray(edge_index),
                  W1=W1, att_src1=att_src1, att_dst1=att_dst1, bias1=bias1,
                  W2=W2, att_src2=att_src2, att_dst2=att_dst2, bias2=bias2)
    out, _ = _run(FULL_CFG, inputs, trace=False)
    return out
